# revision 14
# baseline (speedup 1.0000x reference)
"""Bahdanau attention Trainium2 kernel.

B=32, T=1, S=4096, H=1024. Data-parallel over batch across 8 NeuronCores
(4 batches/core). Per core, a single-pass streaming kernel:

  - weights are cast fp32->bf16 by SWDGE cast-DMAs; Wh/Ws transposed on the
    PE at startup, Wout on the DMA xbar off the critical path
  - encoder tiles [128s, 1024h] stream in via cast-DMA (bf16 natural for the
    context matmul) and through an xbar transpose-DMA (h-major for the score
    matmul)
  - TensorE: h_proj accumulation, inline context matmuls (software-pipelined
    CTX_DELAY tiles behind), final output matmul
  - VectorE: +q_proj bias (broadcast add), fused multiply-reduce against v
  - ScalarE: tanh, per-tile exp with free-dim accumulation for the softmax
    denominator

softmax is computed without max-subtraction: |score| <= ||v||_1 ~ 26, so
exp stays comfortably inside fp32/bf16 range. Context is accumulated
unnormalized and scaled by 1/denom at batch end.

src_lengths is (faithfully to the reference) unused.
"""
import numpy as np
from contextlib import ExitStack

import concourse.bass as bass
import concourse.tile as tile
from concourse import bacc, mybir, masks
from concourse import bass_isa
from concourse import bass_utils

F32 = mybir.dt.float32
BF16 = mybir.dt.bfloat16
Tanh = mybir.ActivationFunctionType.Tanh
Exp = mybir.ActivationFunctionType.Exp
Copy = mybir.ActivationFunctionType.Copy

B, T, S, H = 32, 1, 4096, 1024
NCORES = 8
BL = B // NCORES       # batches per core
NS = S // 128          # s-tiles per batch
NHB = H // 128         # h blocks
NKB = 2 * H // 128     # k blocks of cat=[ctx;query]
ENC_BUFS = 3           # chunks of CH s-tiles each
CH = 4                 # s-tiles per enc DMA chunk
CTX_DELAY = 2          # tiles the inline ctx matmuls trail the score chain


def _build_program():
    nc = bacc.Bacc("TRN2", target_bir_lowering=False, debug=False)

    q_d = nc.dram_tensor("query", (BL, T, H), F32, kind="ExternalInput").ap()
    enc_d = nc.dram_tensor("encoder_outputs", (BL, S, H), F32,
                           kind="ExternalInput").ap()
    ws_d = nc.dram_tensor("Ws_w", (H, H), F32, kind="ExternalInput").ap()
    wh_d = nc.dram_tensor("Wh_w", (H, H), F32, kind="ExternalInput").ap()
    v_d = nc.dram_tensor("v_w", (1, H), F32, kind="ExternalInput").ap()
    wout_d = nc.dram_tensor("Wout_w", (H, 2 * H), F32, kind="ExternalInput").ap()
    out_d = nc.dram_tensor("out", (BL, T, H), F32, kind="ExternalOutput").ap()

    with tile.TileContext(nc) as tc, ExitStack() as ctx:
        # ---------------- pools ----------------
        wt_pool = ctx.enter_context(tc.tile_pool(name="wt", bufs=1))
        wnat_pool = ctx.enter_context(tc.tile_pool(name="wnat", bufs=10))
        wonat_pool = ctx.enter_context(tc.tile_pool(name="wonat", bufs=3))
        enc_pool = ctx.enter_context(tc.tile_pool(name="encp", bufs=ENC_BUFS))
        encT_pool = ctx.enter_context(tc.tile_pool(name="encTp", bufs=3))
        sum_pool = ctx.enter_context(tc.tile_pool(name="sump", bufs=2))
        tanh_pool = ctx.enter_context(tc.tile_pool(name="tanhp", bufs=3))
        junk_pool = ctx.enter_context(tc.tile_pool(name="junkp", bufs=2))
        sc_pool = ctx.enter_context(tc.tile_pool(name="scp", bufs=2))
        small_pool = ctx.enter_context(tc.tile_pool(name="smallp", bufs=1))

        hp_psum = ctx.enter_context(tc.tile_pool(name="hp_ps", bufs=3, space="PSUM"))
        ctx_psum = ctx.enter_context(tc.tile_pool(name="ctx_ps", bufs=1, space="PSUM"))
        tr_psum = hp_psum  # transpose staging shares the hp slots (tag "hp")

        # ---------------- constants ----------------
        id128 = small_pool.tile([128, 128], BF16)
        masks.make_identity(nc, id128[:])
        id4 = small_pool.tile([4, 4], BF16)
        masks.make_identity(nc, id4[:])

        # ---------------- weight casts issued up-front ----------------
        ws_nat, wh_nat = [], []
        for j in range(NHB):
            wN = wnat_pool.tile([128, H], BF16, tag="wnat")
            nc.gpsimd.dma_start(wN[:], wh_d[j * 128:(j + 1) * 128, :])
            wh_nat.append(wN)

        # encoder chunk loader (cast-DMA + xbar transpose), memoized so the
        # first chunks can be issued ahead of the query path
        chunk_tiles = {}

        def chunk_dma(b, c):
            if (b, c) in chunk_tiles:
                return chunk_tiles[(b, c)]
            encN4 = enc_pool.tile([128, CH, H], BF16, tag="encN")
            src = enc_d[b, c * CH * 128:(c + 1) * CH * 128, :]
            nc.gpsimd.dma_start(
                encN4[:], src.rearrange("(t p) h -> p t h", p=128))
            encT4 = encT_pool.tile([128, CH * NHB, 128], BF16, tag="encT")
            nc.sync.dma_start(encT4[:], encN4[:], transpose=True)
            chunk_tiles[(b, c)] = (encN4, encT4)
            return chunk_tiles[(b, c)]

        for c in range(2):
            chunk_dma(0, c)
        for j in range(NHB):
            wN = wnat_pool.tile([128, H], BF16, tag="wnat")
            nc.gpsimd.dma_start(wN[:], ws_d[j * 128:(j + 1) * 128, :])
            ws_nat.append(wN)
        q_sb = small_pool.tile([BL, H], BF16)
        nc.gpsimd.dma_start(q_sb[:], q_d[0:BL, 0, :])
        v_row = small_pool.tile([1, H], BF16)
        nc.gpsimd.dma_start(v_row[:], v_d[0:1, :])
        v_bcast = small_pool.tile([128, H], BF16)
        nc.gpsimd.partition_broadcast(v_bcast[:], v_row[:])
        chunk_dma(0, 2)

        # ---------------- weight transposes on the PE ----------------
        def pe_transpose_wT(nat_tiles, name):
            # nat[j][o, h] (o-block j)  ->  wT[h%128, hb, o] = W[o, h]
            wT = wt_pool.tile([128, NHB, H], BF16, tag=name)
            for j in range(NHB):
                tp = tr_psum.tile([128, NHB * 128], BF16, tag="hp")
                for hb in range(NHB):
                    nc.tensor.transpose(tp[:, hb * 128:(hb + 1) * 128],
                                        nat_tiles[j][:, hb * 128:(hb + 1) * 128],
                                        id128[:])
                nc.vector.tensor_copy(wT[:, :, j * 128:(j + 1) * 128], tp[:])
            return wT

        whT = pe_transpose_wT(wh_nat, "whT")
        wsT = pe_transpose_wT(ws_nat, "wsT")

        # ---------------- query path ----------------
        # catT[k%128, kb, b]: blocks 0..7 = ctx^T (filled later), 8..15 = q^T
        catT = small_pool.tile([128, NKB, BL], BF16)
        qt_ps = tr_psum.tile([128, NHB * BL], BF16, tag="hp")
        for j in range(NHB):
            nc.tensor.transpose(qt_ps[:, j * BL:(j + 1) * BL],
                                q_sb[0:BL, j * 128:(j + 1) * 128], id4[:])
        nc.vector.tensor_copy(catT[:, NHB:2 * NHB, :], qt_ps[:])

        qp_ps = hp_psum.tile([BL, H], F32, tag="hp")
        for hb in range(NHB):
            for half in range(2):
                nc.tensor.matmul(qp_ps[:, half * 512:(half + 1) * 512],
                                 catT[:, NHB + hb, :],
                                 wsT[:, hb, half * 512:(half + 1) * 512],
                                 start=(hb == 0), stop=(hb == NHB - 1))
        qp_sb4 = small_pool.tile([BL, H], BF16)
        nc.scalar.copy(qp_sb4[:], qp_ps[:])
        qp_flat = small_pool.tile([1, BL * H], BF16)
        for b in range(BL):
            nc.sync.dma_start(qp_flat[0:1, b * H:(b + 1) * H], qp_sb4[b:b + 1, :])
        # broadcast q_proj rows across partitions for the per-tile bias add
        # (ones-matmul broadcast; keeps the gpsimd DGE queue free)
        ones_row = small_pool.tile([1, 128], BF16)
        nc.gpsimd.memset(ones_row[:], 1.0)
        qp_bcast = small_pool.tile([128, BL, H], BF16)
        for b in range(BL):
            qb_ps = hp_psum.tile([128, H], F32, tag="hp")
            for half in range(2):
                nc.tensor.matmul(qb_ps[:, half * 512:(half + 1) * 512],
                                 ones_row[:],
                                 qp_flat[0:1, b * H + half * 512:
                                         b * H + (half + 1) * 512])
            nc.scalar.copy(qp_bcast[:, b, :], qb_ps[:])

        ctx4_sb = small_pool.tile([BL, H], BF16)

        # ---------------- main loop ----------------
        for b in range(BL):
            scores = sc_pool.tile([128, NS], F32, tag="scores")
            attnU = sc_pool.tile([128, NS], BF16, tag="attnU")
            dcol = sc_pool.tile([128, NS], F32, tag="dcol")
            ctx_ps = ctx_psum.tile([1, H], F32)
            enc_tiles = [None] * NS
            tanh_tiles = [None] * NS

            def emit_score_tail(st):
                junk = junk_pool.tile([128, H], BF16, tag="junk")
                nc.vector.scalar_tensor_tensor(
                    out=junk[:], in0=tanh_tiles[st][:], scalar=1.0,
                    in1=v_bcast[:],
                    op0=mybir.AluOpType.mult, op1=mybir.AluOpType.mult,
                    accum_out=scores[:, st:st + 1])
                nc.scalar.activation(attnU[:, st:st + 1], scores[:, st:st + 1],
                                     Exp, accum_out=dcol[:, st:st + 1])

            def emit_ctx_mm(st):
                for half in range(2):
                    nc.tensor.matmul(ctx_ps[:, half * 512:(half + 1) * 512],
                                     attnU[:, st:st + 1],
                                     enc_tiles[st][:, half * 512:(half + 1) * 512],
                                     start=(st == 0), stop=(st == NS - 1))

            for st in range(NS):
                t = st % CH
                if t == 0:
                    encN4, encT4 = chunk_dma(b, st // CH)
                enc_tiles[st] = encN4[:, t, :]

                hp = hp_psum.tile([128, H], F32, tag="hp")
                for hb in range(NHB):
                    for half in range(2):
                        nc.tensor.matmul(hp[:, half * 512:(half + 1) * 512],
                                         encT4[:, t * NHB + hb, :],
                                         whT[:, hb, half * 512:(half + 1) * 512],
                                         start=(hb == 0), stop=(hb == NHB - 1))
                # trailing ctx matmuls (software pipeline)
                if st >= CTX_DELAY:
                    emit_ctx_mm(st - CTX_DELAY)

                sum_sb = sum_pool.tile([128, H], BF16, tag="sum")
                nc.vector.tensor_add(sum_sb[:], hp[:], qp_bcast[:, b, :])
                tanh_sb = tanh_pool.tile([128, H], BF16, tag="tanh")
                nc.scalar.activation(tanh_sb[:], sum_sb[:], Tanh)
                tanh_tiles[st] = tanh_sb
                if st >= 1:
                    emit_score_tail(st - 1)
            emit_score_tail(NS - 1)
            for st in range(NS - CTX_DELAY, NS):
                emit_ctx_mm(st)

            # denominator & 1/denom
            dsum = sc_pool.tile([128, 1], F32, tag="dsum")
            nc.vector.reduce_sum(dsum[:], dcol[:], axis=mybir.AxisListType.X)
            d_all = sc_pool.tile([128, 1], F32, tag="dall")
            nc.gpsimd.partition_all_reduce(d_all[:], dsum[:], channels=128,
                                           reduce_op=bass_isa.ReduceOp.add)
            inv_d = sc_pool.tile([1, 1], F32, tag="invd")
            nc.vector.reciprocal(inv_d[:], d_all[0:1, :])

            # normalize by 1/denom while copying out, move to partition b
            ctx_row = sc_pool.tile([1, H], BF16, tag="ctxrow")
            nc.scalar.activation(ctx_row[:], ctx_ps[:], Copy,
                                 scale=inv_d[0:1, 0:1])
            nc.sync.dma_start(ctx4_sb[b:b + 1, :], ctx_row[:])

        # ---------------- Wout prep (off critical path, xbar) -------------
        woutT = wt_pool.tile([128, NKB, H], BF16, tag="woutT")
        for j in range(NHB):
            with tc.tile_wait_until(0.20 + j * 0.02):
                woN = wonat_pool.tile([128, 2 * H], BF16, tag="wonat")
                nc.gpsimd.dma_start(woN[:], wout_d[j * 128:(j + 1) * 128, :])
            with tc.tile_wait_until(0.26 + j * 0.02):
                nc.sync.dma_start(woutT[:, :, j * 128:(j + 1) * 128], woN[:],
                                  transpose=True)

        # ---------------- finale ----------------
        # query half of the output matmul first (doesn't need ctx)
        out_ps = hp_psum.tile([BL, H], F32, tag="hp")
        for kb in range(NHB, NKB):
            for half in range(2):
                nc.tensor.matmul(out_ps[:, half * 512:(half + 1) * 512],
                                 catT[:, kb, :],
                                 woutT[:, kb, half * 512:(half + 1) * 512],
                                 start=(kb == NHB), stop=False)
        ct_ps = tr_psum.tile([128, NHB * BL], BF16, tag="hp")
        for j in range(NHB):
            nc.tensor.transpose(ct_ps[:, j * BL:(j + 1) * BL],
                                ctx4_sb[0:BL, j * 128:(j + 1) * 128], id4[:])
        nc.vector.tensor_copy(catT[:, 0:NHB, :], ct_ps[:])
        for kb in range(NHB):
            for half in range(2):
                nc.tensor.matmul(out_ps[:, half * 512:(half + 1) * 512],
                                 catT[:, kb, :],
                                 woutT[:, kb, half * 512:(half + 1) * 512],
                                 start=False, stop=(kb == NHB - 1))
        out_sb = small_pool.tile([BL, H], F32)
        nc.scalar.activation(out_sb[:], out_ps[:], Tanh)
        nc.sync.dma_start(out_d[0:BL, 0, :], out_sb[:])

    nc.compile()
    return nc


_program = None


def get_program():
    global _program
    if _program is None:
        _program = _build_program()
    return _program


def run_sharded(inputs, trace=False, **kw):
    nc = get_program()
    in_maps = []
    for i in range(NCORES):
        sl = slice(i * BL, (i + 1) * BL)
        in_maps.append({
            "query": np.ascontiguousarray(inputs["query"][sl], dtype=np.float32),
            "encoder_outputs": np.ascontiguousarray(
                inputs["encoder_outputs"][sl], dtype=np.float32),
            "Ws_w": np.asarray(inputs["Ws_w"], dtype=np.float32),
            "Wh_w": np.asarray(inputs["Wh_w"], dtype=np.float32),
            "v_w": np.asarray(inputs["v_w"], dtype=np.float32),
            "Wout_w": np.asarray(inputs["Wout_w"], dtype=np.float32),
        })
    res = bass_utils.run_bass_kernel_spmd(
        nc, in_maps, core_ids=list(range(NCORES)), trace=trace, **kw)
    out = np.concatenate(
        [np.asarray(res.results[i]["out"], dtype=np.float32).reshape(BL, T, H)
         for i in range(NCORES)], axis=0)
    return out, res


def kernel(**inputs):
    out, _ = run_sharded(inputs)
    return out
